# revision 30
# baseline (speedup 1.0000x reference)
"""Trainium2 Bass kernel for nn_ArithmeticExperts (reciprocal_table).

Reference math per element:
    sign = sign(x); xa = |x|
    exp  = floor(log2(xa)) + 1 ; temp = xa * 2^-exp  (mantissa in [0.5, 1))
    idx  = (temp - 0.5) * 256
    y0   = softmax(-|arange(256) - idx| * 1000) @ table   # sharp softmax
    y    = y0*(2 - temp*y0); y = y*(2 - temp*y)           # 2 Newton steps
    out  = y * 2^-exp * sign

Implementation notes:
  - In f32 the scale-1000 softmax collapses to a 2-neighbor blend; nearest-
    neighbor rounding instead changes y0 by <=2e-3 relative, damped by the
    Newton steps to <5e-4 on the final result (validated: max rel 4.9e-4 vs
    the jax reference on the real inputs).
  - table[j] == f32(1/(0.5 + j/512)), so the lookup is arithmetic:
    u  = RN(temp*256 + (2^23-128)) = 2^23 + round(idx)   (magic rounding)
    tx = u/512 - 16383.5           = 0.5 + round(idx)/512
    y0 = 1/tx  via the DVE's IEEE-exact Reciprocal == the table value.
  - exp/temp/sign are extracted with int32 bit ops; the final *2^-exp*sign
    is one exact float multiply by ss = sign*2^-exp, built from the halved
    exponent field (DVE/Pool int32 arithmetic saturates; the halving keeps
    every intermediate in range).
  - Engine split: GPSIMD does the int front-end (tempn, es3, ss) and the
    final multiply; ACT does the two affine/rounding Copies; DVE does
    reciprocal + the two Newton steps (5 ops).  DMAs ride SP's HWDGE.
  - Pure data parallel: 8 cores x 65536 contiguous elements, no collectives.
  - Raw Bass (no TileContext): this container's walrus allows only 1 sync
    wait per DMA instruction; Tile's kernel-tail drain violates that, so
    synchronization is manual with standalone waits.
  - Semaphores are cleared by their last waiters so a loaded NEFF can be
    re-executed.
"""

import sys

if "/opt/trn_rl_repo" not in sys.path:
    sys.path.insert(0, "/opt/trn_rl_repo")

import numpy as np

N = 524288
N_CORES = 8
SHARD = N // N_CORES          # 65536
P = 128
F = SHARD // P                # 512 elements per partition
N_TILES = 2

_MANT = 0x007FFFFF                      # mantissa mask
_NEG_HALF_EXP = 0xBF000000 - (1 << 32)  # sign+exponent of -0.5 (signed int32)
_SIGNEXP = 0xFF800000 - (1 << 32)       # sign+exponent mask (signed int32)
_HALF_C2 = 0x3F400000                   # (0x7E800000)/2: half the scale rebias


def _build_bass(n_tiles=N_TILES, gp_front=True, gp_out=True, final_wait=True,
                end_mode="none", reps=1, tile_cols=None, out_dma="sync",
                t0_dve=False, gp_ss=False, recip_mode="hw"):
    import contextlib

    import concourse.bass as bass
    import concourse.mybir as mybir
    from concourse.alu_op_type import AluOpType

    f32 = mybir.dt.float32
    i32 = mybir.dt.int32
    if tile_cols is None:
        tile_cols = [F // n_tiles] * n_tiles
    else:
        tile_cols = list(tile_cols)
        n_tiles = len(tile_cols)
    assert sum(tile_cols) == F, tile_cols
    tile_off = [sum(tile_cols[:i]) for i in range(n_tiles)]
    Copy = mybir.ActivationFunctionType.Copy

    # Bass.__init__ emits 4 const-AP memsets + an all-engine startup
    # barrier (~1us before the first DMA can issue). This kernel uses no
    # const APs, so skip both during construction.
    _orig_barrier = bass.Bass.all_engine_barrier
    _orig_memset = bass.BassSharedVectorInterface.memset
    bass.Bass.all_engine_barrier = lambda self, **kw: None
    bass.BassSharedVectorInterface.memset = lambda self, ap, c: None
    try:
        nc = bass.Bass(trn_type="TRN2")
    finally:
        bass.Bass.all_engine_barrier = _orig_barrier
        bass.BassSharedVectorInterface.memset = _orig_memset
    x_d = nc.dram_tensor("x", [P, F], f32, kind="ExternalInput")
    o_d = nc.dram_tensor("out", [P, F], f32, kind="ExternalOutput")

    with contextlib.ExitStack() as st:
        ent = st.enter_context
        xt = ent(nc.sbuf_tensor([P, F], f32))
        tempt = ent(nc.sbuf_tensor([P, F], f32))
        est = ent(nc.sbuf_tensor([P, F], i32))
        sst = ent(nc.sbuf_tensor([P, F], f32))
        ut = ent(nc.sbuf_tensor([P, F], f32))
        txt = ent(nc.sbuf_tensor([P, F], f32))
        y0t = ent(nc.sbuf_tensor([P, F], f32))
        nxt = ent(nc.sbuf_tensor([P, F], i32))
        y0at = ent(nc.sbuf_tensor([P, F], f32))
        pt = ent(nc.sbuf_tensor([P, F], f32))
        q1t = ent(nc.sbuf_tensor([P, F], f32))
        q2t = ent(nc.sbuf_tensor([P, F], f32))
        y1t = ent(nc.sbuf_tensor([P, F], f32))
        y2t = ent(nc.sbuf_tensor([P, F], f32))
        ot = ent(nc.sbuf_tensor([P, F], f32))

        s_in = ent(nc.semaphore(name="s_in"))    # input DMA done (16/tile)
        s_gpf = ent(nc.semaphore(name="s_gpf"))  # front-end done (1/tile)
        s_act = ent(nc.semaphore(name="s_act"))  # ACT tx done (1/tile)
        s_dve = ent(nc.semaphore(name="s_dve"))  # DVE y2 done (1/tile)
        s_gpo = ent(nc.semaphore(name="s_gpo"))  # final mult done (1/tile)
        s_od = ent(nc.semaphore(name="s_od"))    # output DMA done (16/tile)

        blk = bass.BassBlock(nc, "blk")
        if end_mode == "barrier":
            ent(blk)
        else:
            blk.__enter__()

        def col(t, i):
            return t[:, tile_off[i]:tile_off[i] + tile_cols[i]]

        front_eng = nc.gpsimd if gp_front else nc.vector
        out_eng = nc.gpsimd if gp_out else nc.vector

        def front_temp(i):
            # tempn = bitcast((x & mant) | -0.5-exponent) = -temp
            front_eng.tensor_scalar(
                col(tempt, i).bitcast(i32), col(xt, i).bitcast(i32),
                _MANT, _NEG_HALF_EXP,
                AluOpType.bitwise_and, AluOpType.bitwise_or,
            ).then_inc(s_gpf, 1)

        def front_es3(i, eng, sem=None):
            # es3 = (x & signexp) >> 1   (halved: int32 arithmetic saturates;
            # bitwise ops are DVE-only, Pool's ucode rejects them)
            ins = eng.tensor_scalar(
                col(est, i), col(xt, i).bitcast(i32),
                _SIGNEXP, 1,
                AluOpType.bitwise_and, AluOpType.logical_shift_right,
            )
            if sem is not None:
                ins.then_inc(sem, 1)

        def front_ss(i, eng, sem=None):
            # ss_bits = (es3 - 0x3F400000)*(-2) = 0x7E800000 - (x&signexp)
            # mod 2^32 -> ss = sign*2^-exp ; intermediates stay in int32 range
            ins = eng.tensor_scalar(
                col(sst, i).bitcast(i32), col(est, i),
                _HALF_C2, -2,
                AluOpType.subtract, AluOpType.mult,
            )
            if sem is not None:
                ins.then_inc(sem, 1)

        def front(i):
            front_temp(i)
            if gp_ss:
                front_es3(i, front_eng, sem=s_es)
            else:
                front_es3(i, front_eng)
                front_ss(i, front_eng)

        def final_mult(i):
            # out = y2 * (sign*2^-exp): exact power-of-two multiply
            out_eng.tensor_mul(
                col(ot, i), col(y2t, i), col(sst, i),
            ).then_inc(s_gpo, 1)

        T = n_tiles

        @blk.sync
        def _(sync):
            for r in range(reps):
                if r > 0:
                    # WAR: iteration r's input DMAs overwrite xt, whose last
                    # readers are iteration r-1's front ops
                    sync.wait_ge(s_gpf, T * r)
                for i in range(T):
                    sync.dma_start(col(xt, i), col(x_d, i)).then_inc(s_in, 16)
                if out_dma == "sync":
                    for i in range(T):
                        sync.wait_ge(s_gpo, T * r + i + 1)
                        sync.dma_start(col(o_d, i), col(ot, i)).then_inc(s_od, 16)
            if out_dma == "sync":
                sync.sem_clear(s_gpo)
                if final_wait:
                    sync.wait_ge(s_od, 16 * T * reps)
                    sync.sem_clear(s_od)

        s_es = ent(nc.semaphore(name="s_es"))    # DVE es3 done (1/tile)
        s_ss = ent(nc.semaphore(name="s_ss"))    # GP ss done (1/tile)

        if gp_ss:
            @blk.gpsimd
            def _(gpsimd):
                for r in range(reps):
                    for i in range(T):
                        gpsimd.wait_ge(s_es, T * r + i + 1)
                        front_ss(i, nc.gpsimd, sem=s_ss)
                    if gp_out:
                        for i in range(T):
                            gpsimd.wait_ge(s_dve, T * r + i + 1)
                            final_mult(i)
                gpsimd.sem_clear(s_es)
                gpsimd.sem_clear(s_ss)
                if gp_out:
                    gpsimd.sem_clear(s_dve)

        if (gp_front or gp_out) and not gp_ss:
            @blk.gpsimd
            def _(gpsimd):
                for r in range(reps):
                    b = T * r
                    if gp_front:
                        if r > 0:
                            # WAR: front overwrites tempt, last read by
                            # iteration r-1's DVE Newton ops
                            gpsimd.wait_ge(s_gpo, b)
                        gpsimd.wait_ge(s_in, 16 * (b + 1))
                        front(0)
                        for i in range(1, T):
                            gpsimd.wait_ge(s_in, 16 * (b + i + 1))
                            front(i)
                            if gp_out:
                                gpsimd.wait_ge(s_dve, b + i)
                                if r > 0:
                                    gpsimd.wait_ge(s_od, 16 * (b - T + i))
                                final_mult(i - 1)
                        if gp_out:
                            gpsimd.wait_ge(s_dve, b + T)
                            if r > 0:
                                gpsimd.wait_ge(s_od, 16 * b)
                            final_mult(T - 1)
                    else:
                        for i in range(T):
                            gpsimd.wait_ge(s_dve, b + i + 1)
                            if r > 0:
                                gpsimd.wait_ge(s_od, 16 * (b - T + i + 1))
                            final_mult(i)
                if gp_front:
                    gpsimd.sem_clear(s_in)
                if gp_out:
                    gpsimd.sem_clear(s_dve)

        # with t0_dve, ACT handles tiles 1..T-1 only; s_act counts once per
        # ACT tile, so back(i) waits s_act >= i
        act_tiles = list(range(1, n_tiles)) if t0_dve else list(range(n_tiles))

        def act_thresh(i):
            return i if t0_dve else i + 1

        @blk.vector
        def _(vector):
          for r in range(reps):
            b = len(act_tiles) * r
            if not gp_front:
                for i in range(n_tiles):
                    vector.wait_ge(s_in, 16 * (T * r + i + 1))
                    front(i)
                    if t0_dve and i == 0:
                        # tile 0's affine/rounding pair inline on DVE: the
                        # back chain starts without the ACT round-trip
                        nc.vector.tensor_scalar(
                            col(ut, 0), col(tempt, 0), -256.0, 8388480.0,
                            AluOpType.mult, AluOpType.add,
                        )
                        nc.vector.tensor_scalar(
                            col(txt, 0), col(ut, 0), 1.0 / 512.0, -16383.5,
                            AluOpType.mult, AluOpType.add,
                        )

            for i in range(n_tiles):
                if not (t0_dve and i == 0):
                    vector.wait_ge(s_act, b + act_thresh(i))
                if recip_mode == "hw":
                    # y0 = 1/tx exactly (trn2 Reciprocal is IEEE 1/x) ==
                    # the table value. NOTE: the HW iterative divide runs
                    # ~5x slower than a single-pass DVE op on silicon.
                    nc.vector.reciprocal(col(y0t, i), col(txt, i))
                else:
                    # y0 ~= 1/tx via bitwise-NOT magic seed + one chebyshev
                    # Newton step (~1.7e-3 rel err; the two reference Newton
                    # steps damp it by ~0.2x on the final result). All ops
                    # run at full DVE rate.
                    nc.vector.tensor_scalar(
                        col(nxt, i), col(txt, i).bitcast(i32),
                        -1, None, AluOpType.bitwise_xor,
                    )
                    nc.vector.tensor_scalar(
                        col(y0at, i), col(nxt, i).bitcast(f32),
                        -0.23549792, None, AluOpType.mult,
                    )
                    nc.vector.tensor_mul(col(pt, i), col(txt, i), col(y0at, i))
                    # y0n = (p - c1)*y0a = -(c1 - p)*y0a = -y0
                    nc.vector.scalar_tensor_tensor(
                        col(y0t, i), col(pt, i), 2.0017324, col(y0at, i),
                        AluOpType.subtract, AluOpType.mult,
                    )
                # Newton 1: y1 = (2 - temp*y0)*y0
                nc.vector.tensor_mul(col(q1t, i), col(tempt, i), col(y0t, i))
                nc.vector.scalar_tensor_tensor(
                    col(y1t, i), col(q1t, i), 2.0, col(y0t, i),
                    AluOpType.add if recip_mode == "hw" else AluOpType.subtract,
                    AluOpType.mult,
                )
                # Newton 2: y2 = (tempn*y1 + 2)*y1
                nc.vector.tensor_mul(col(q2t, i), col(tempt, i), col(y1t, i))
                ins_y2 = nc.vector.scalar_tensor_tensor(
                    col(y2t, i), col(q2t, i), 2.0, col(y1t, i),
                    AluOpType.add, AluOpType.mult,
                )
                if gp_out:
                    ins_y2.then_inc(s_dve, 1)
                else:
                    if r > 0:
                        # WAR: final_mult overwrites ot, read by iteration
                        # r-1's output DMA
                        vector.wait_ge(s_od, 16 * (b - T + i + 1))
                    if gp_ss:
                        vector.wait_ge(s_ss, T * r + i + 1)
                    final_mult(i)
          if not gp_front:
              vector.sem_clear(s_in)
          vector.sem_clear(s_act)

        @blk.scalar
        def _(scalar):
            for r in range(reps):
                b = T * r
                for i in act_tiles:
                    scalar.wait_ge(s_gpf, b + i + 1)
                    if r > 0:
                        # WAR: overwrites ut/txt, last read by iteration r-1's
                        # DVE reciprocal (ordered before its s_gpo inc)
                        scalar.wait_ge(s_gpo, b - T + i + 1)
                    # u = RN(temp*256 + (2^23 - 128)) = 2^23 + round(idx)
                    nc.scalar.activation(
                        col(ut, i), col(tempt, i), Copy, bias=8388480.0, scale=-256.0,
                    )
                    # tx = u/512 - 16383.5 = 0.5 + round(idx)/512
                    nc.scalar.activation(
                        col(txt, i), col(ut, i), Copy, bias=-16383.5, scale=1.0 / 512.0,
                    ).then_inc(s_act, 1)
                if out_dma == "scalar":
                    for i in range(n_tiles):
                        scalar.wait_ge(s_gpo, b + i + 1)
                        scalar.dma_start(col(o_d, i), col(ot, i)).then_inc(s_od, 16)
            scalar.sem_clear(s_gpf)
            if out_dma == "scalar":
                scalar.sem_clear(s_gpo)
                if final_wait:
                    scalar.wait_ge(s_od, 16 * T * reps)
                    scalar.sem_clear(s_od)

        if end_mode != "barrier":
            for engine, last_body in blk.last_body.items():
                with nc.body(
                    last_body, parent=nc.cur_bb, allow_existing_parent=True
                ):
                    engine.br(blk.end_bb)
            nc.switch_bb(blk.end_bb)
            if end_mode == "drains":
                for eng_type, eng in nc.engines.items():
                    d = mybir.InstDrain(
                        name=nc.get_next_instruction_name(),
                        ins=[], outs=[], bass_is_fusable=False,
                    )
                    d.engine = eng_type
                    eng.add_instruction(d)

    return nc


_CACHED = {}


def _get_nc(**kw):
    key = tuple(sorted(kw.items()))
    if key not in _CACHED:
        _CACHED[key] = _build_bass(**dict(key))
    return _CACHED[key]


# recip_mode="seed": the HW Reciprocal (iterative divide) runs ~5x slower
# than a single-pass DVE op on real silicon (documented in bass.py); the
# magic-seed + chebyshev-Newton path uses only full-rate standard ops and
# costs 8.2e-4 max rel err vs the reference (gate is 2e-2).
BEST_CONFIG = dict(
    tile_cols=(192, 320), gp_front=False, gp_out=False, out_dma="sync",
    final_wait=True, end_mode="drains", t0_dve=True, gp_ss=False,
    recip_mode="seed",
)


def kernel(x: np.ndarray, recip_table_val: np.ndarray = None, **_unused) -> np.ndarray:
    from concourse.bass_utils import run_bass_kernel_spmd

    x = np.ascontiguousarray(np.asarray(x, dtype=np.float32))
    assert x.shape == (N,), x.shape

    nc = _get_nc(**BEST_CONFIG)
    in_maps = [
        {"x": x[i * SHARD:(i + 1) * SHARD].reshape(P, F)} for i in range(N_CORES)
    ]
    res = run_bass_kernel_spmd(nc, in_maps, core_ids=list(range(N_CORES)))
    outs = [res.results[i]["out"].reshape(SHARD) for i in range(N_CORES)]
    return np.concatenate(outs).astype(np.float32)


if __name__ == "__main__":
    rng = np.random.default_rng(0)
    x = (rng.uniform(1.0, 1000.0, N) * np.where(rng.random(N) < 0.5, 1.0, -1.0)).astype(np.float32)
    y = kernel(x)
    print("ok", y[:4], 1.0 / x[:4])
